# revision 1
# baseline (speedup 1.0000x reference)
"""Ragged-sequence multi-head attention (B=16, S=1024, D=512, H=8, DH=64)
for 8 Trainium2 NeuronCores.

Strategy: data-parallel over the batch. The 16 sequences are sorted by
length; the 8 longest go to slot 0 (one per core), the 8 shortest to
slot 1. A single SPMD Bass program processes both slots with per-slot
static loop bounds equal to ceil128(max length in that slot); within a
bound, invalid key positions are masked via a per-partition additive
bias on the exp() activation, and padded query rows are zeroed via a
per-partition multiplicative mask.

Host-side prep (inside kernel()): x rows are gathered per core and
pre-TRANSPOSED to feature-major fp16 (no PE transposes on device);
weights are pre-cast fp16 and pre-rearranged to [128, 4, 512] (no
on-device staging casts).

Per-core pipeline (per slot):
  1. QT = Wq^T @ x^T, KT likewise (feature-major), V in [s, d] layout
  2. per head-pair, per q-chunk, per k-tile:
       scoresT[k, q] = K^T q   (row-packed head pair on the PE array)
       expT = exp(0.125 * scoresT + key_mask_bias)   (ACT engine)
       outT[d, q]  += V^T expT (col-packed head pair)
       denom[., q] += 1^T expT (col-packed head pair, replicated rows)
  3. outT_norm = outT * reciprocal(denom)   (DVE)
  4. out[s, d] = outT_norm^T @ Wo + bo, masked by query validity
"""

import math
import os

import numpy as np

B, S, D = 16, 1024, 512
H, DH = 8, 64
N_CORES = 8
P = 128  # partitions
KC = D // P  # 4 contraction chunks of 128
NT_MAX = S // P  # 8 key tiles max

_BUILD_CACHE: dict = {}


def _ceil128(n: int) -> int:
    return max(P, (int(n) + P - 1) // P * P)


def _build_bass(bounds: tuple[int, int]):
    """Build the Bass program for per-slot bounds (multiples of 128)."""
    from contextlib import ExitStack

    import concourse.bass as bass
    import concourse.mybir as mybir
    import concourse.tile as tile
    from concourse import bacc

    fp32 = mybir.dt.float32
    fp16 = mybir.dt.float16
    Exp = mybir.ActivationFunctionType.Exp
    mult = mybir.AluOpType.mult
    add = mybir.AluOpType.add

    nc = bacc.Bacc("TRN2", target_bir_lowering=False, debug=False)

    xt_d = [
        nc.dram_tensor(f"xt{b}", [P, KC, bounds[b]], fp16, kind="ExternalInput").ap()
        for b in (0, 1)
    ]
    kbias_d = nc.dram_tensor("kbias", [2, P, NT_MAX], fp32, kind="ExternalInput").ap()
    w_d = {
        name: nc.dram_tensor(name, [P, KC, D], fp16, kind="ExternalInput").ap()
        for name in ("wq", "wk", "wv", "wo")
    }
    bo_d = nc.dram_tensor("bo", [D], fp32, kind="ExternalInput").ap()
    out_d = nc.dram_tensor("out", [2, S, D], fp16, kind="ExternalOutput").ap()

    NT = [bounds[0] // P, bounds[1] // P]
    QCH = [
        [(qs, min(512, bounds[b] - qs)) for qs in range(0, bounds[b], 512)]
        for b in (0, 1)
    ]

    with ExitStack() as ctx:
        tc = ctx.enter_context(tile.TileContext(nc))
        singles = ctx.enter_context(tc.tile_pool(name="singles", bufs=1))
        big = ctx.enter_context(tc.tile_pool(name="big", bufs=1))
        epool = ctx.enter_context(tc.tile_pool(name="epool", bufs=3))
        opool = ctx.enter_context(tc.tile_pool(name="opool", bufs=4))
        mmps = ctx.enter_context(tc.tile_pool(name="mmps", bufs=2, space="PSUM"))
        scps = ctx.enter_context(tc.tile_pool(name="scps", bufs=2, space="PSUM"))
        accps = ctx.enter_context(tc.tile_pool(name="accps", bufs=1, space="PSUM"))

        # ---- weights / constants (fp16, pre-arranged on host) ----
        ones64 = singles.tile([P, DH], fp16)
        nc.vector.memset(ones64, 1.0)
        w_sb = {}
        for name in ("wv", "wq", "wk", "wo"):
            w_sb[name] = singles.tile([P, KC, D], fp16, name=f"w_{name}")

        def load_weight(name):
            nc.sync.dma_start(out=w_sb[name], in_=w_d[name])

        # weights ride the sync queue; x^T chunks ride the gpsimd queue
        # in parallel, ordered by first use (V slot0 needs wv + xT0).
        load_weight("wv")

        # ---- x^T: direct chunked DMA (pre-transposed on host) ----
        xT = []
        for b in (0, 1):
            xT.append(big.tile([P, KC, bounds[b]], fp16, name=f"xT{b}", tag=f"xT{b}"))
        for b in (0, 1):
            for cs in range(0, bounds[b], 512):
                w = min(512, bounds[b] - cs)
                nc.gpsimd.dma_start(
                    out=xT[b][:, :, cs : cs + w], in_=xt_d[b][:, :, cs : cs + w]
                )

        for name in ("wq", "wk", "wo"):
            load_weight(name)
        kbias_sb = singles.tile([P, 2, NT_MAX], fp32)
        nc.sync.dma_start(out=kbias_sb, in_=kbias_d.rearrange("b p t -> p b t"))
        bo_rep = singles.tile([P, D], fp32)
        bo_bcast = bass.AP(tensor=bo_d.tensor, offset=bo_d.offset, ap=[[0, P], [1, D]])
        nc.gpsimd.dma_start(out=bo_rep, in_=bo_bcast)

        # ---- V: slot 0 emitted now; slot 1 rides the filler ----
        V = [
            big.tile([P, NT[b], D], fp16, name=f"V{b}", tag=f"V{b}")
            for b in (0, 1)
        ]

        def v_units(b, st):
            ps_box = []

            def mk_mm(kc):
                def emit():
                    if not ps_box:
                        ps_box.append(
                            mmps.tile([P, 512], fp32, name="v_ps", tag="mm")
                        )
                    nc.tensor.matmul(
                        ps_box[0],
                        xT[b][:, kc, st * P : (st + 1) * P],
                        w_sb["wv"][:, kc, :],
                        start=(kc == 0),
                        stop=(kc == KC - 1),
                    )
                return emit

            def fin():
                nc.vector.tensor_copy(out=V[b][:, st, :], in_=ps_box[0])

            return [mk_mm(kc) for kc in range(KC)] + [fin]

        for st in range(NT[0]):
            for u in v_units(0, st):
                u()

        QT = [
            big.tile([P, KC, bounds[b]], fp16, name=f"QT{b}", tag=f"QT{b}")
            for b in (0, 1)
        ]
        KT = [
            big.tile([P, KC, bounds[b]], fp16, name=f"KT{b}", tag=f"KT{b}")
            for b in (0, 1)
        ]
        outT = [
            big.tile([P, KC, bounds[b]], fp16, name=f"oT{b}", tag=f"oT{b}")
            for b in (0, 1)
        ]

        def qtkt_units(b, hp, dst, wname, qs, w):
            ps_box = []

            def mk_mm(kc):
                def emit():
                    if not ps_box:
                        ps_box.append(
                            mmps.tile([P, 512], fp32, name="qk_ps", tag="mm")
                        )
                    nc.tensor.matmul(
                        ps_box[0][:, :w],
                        w_sb[wname][:, kc, hp * P : (hp + 1) * P],
                        xT[b][:, kc, qs : qs + w],
                        start=(kc == 0),
                        stop=(kc == KC - 1),
                    )
                return emit

            def fin():
                # split copy into partition halves so RAW deps register
                # against the score matmuls' half-partition reads
                nc.vector.tensor_copy(
                    out=dst[0:DH, hp, qs : qs + w], in_=ps_box[0][0:DH, :w]
                )
                nc.vector.tensor_copy(
                    out=dst[DH:P, hp, qs : qs + w], in_=ps_box[0][DH:P, :w]
                )

            return [mk_mm(kc) for kc in range(KC)] + [fin]

        def outproj_units(b, st):
            ps_box = []

            def mk_mm(hc):
                def emit():
                    if not ps_box:
                        ps_box.append(
                            mmps.tile([P, 512], fp32, name="fo_ps", tag="mm")
                        )
                    nc.tensor.matmul(
                        ps_box[0],
                        outT[b][:, hc, st * P : (st + 1) * P],
                        w_sb["wo"][:, hc, :],
                        start=(hc == 0),
                        stop=(hc == KC - 1),
                    )
                return emit

            def fin():
                fout = opool.tile([P, D], fp16, tag="fout")
                nc.vector.tensor_tensor(fout, ps_box[0], bo_rep, add)
                nc.sync.dma_start(
                    out=out_d[b, st * P : (st + 1) * P, :], in_=fout
                )

            return [mk_mm(hc) for hc in range(KC)] + [fin]

        def attn_chunk(b, hp, qs, w, filler, iters_left):
            o_ps = accps.tile([P, 512], fp32, name="o_ps", tag="o_ps")
            d_ps = accps.tile([P, 512], fp32, name="d_ps", tag="d_ps")
            nt = NT[b]

            def emit_scores_exp(kt):
                s_pair = scps.tile([P, 1024], fp32, name="s_pair", tag="s_pair")
                nc.tensor.matmul(
                    s_pair[:, 0:w],
                    KT[b][0:DH, hp, kt * P : (kt + 1) * P],
                    QT[b][0:DH, hp, qs : qs + w],
                    start=True, stop=True, tile_position=(0, 0),
                )
                nc.tensor.matmul(
                    s_pair[:, 512 : 512 + w],
                    KT[b][DH:P, hp, kt * P : (kt + 1) * P],
                    QT[b][DH:P, hp, qs : qs + w],
                    start=True, stop=True, tile_position=(DH, 0),
                )
                e_pair = epool.tile([P, 2, 512], fp16, name="e_pair", tag="e_pair")
                nc.scalar.activation(
                    e_pair[:, :, :w],
                    s_pair.rearrange("p (h q) -> p h q", h=2)[:, :, :w],
                    Exp, bias=kbias_sb[:, b, kt : kt + 1], scale=DH**-0.5,
                )
                return e_pair

            def emit_pv(kt, e_pair):
                first, last = kt == 0, kt == nt - 1
                nc.tensor.matmul(
                    o_ps[0:DH, :w], V[b][:, kt, hp * P : hp * P + DH],
                    e_pair[:, 0, :w], start=first, stop=last,
                    tile_position=(0, 0), skip_group_check=True,
                )
                nc.tensor.matmul(
                    o_ps[DH:P, :w], V[b][:, kt, hp * P + DH : (hp + 1) * P],
                    e_pair[:, 1, :w], start=first, stop=last,
                    tile_position=(0, DH), skip_group_check=True,
                )
                nc.tensor.matmul(
                    d_ps[0:DH, :w], ones64, e_pair[:, 0, :w],
                    start=first, stop=last,
                    tile_position=(0, 0), skip_group_check=True,
                )
                nc.tensor.matmul(
                    d_ps[DH:P, :w], ones64, e_pair[:, 1, :w],
                    start=first, stop=last,
                    tile_position=(0, DH), skip_group_check=True,
                )

            pending = None
            for kt in range(nt):
                e_pair = emit_scores_exp(kt)
                if pending is not None:
                    emit_pv(*pending)
                pending = (kt, e_pair)
                if filler and iters_left[0] > 0:
                    k = -(-len(filler) // iters_left[0])
                    for _ in range(min(k, len(filler))):
                        filler.pop(0)()
                iters_left[0] -= 1
            emit_pv(*pending)
            rrep = epool.tile([P, 512], fp32, tag="rrep", bufs=2)
            nc.vector.reciprocal_approx_fast(out=rrep[:, :w], in_=d_ps[:, :w])
            nc.vector.tensor_tensor(
                outT[b][:, hp, qs : qs + w], o_ps[:, :w], rrep[:, :w], mult
            )

        # ---- choreographed emission ----
        for dst, wname in ((QT[0], "wq"), (KT[0], "wk")):
            for qs, w in QCH[0]:
                for u in qtkt_units(0, 0, dst, wname, qs, w):
                    u()

        blocks = [(0, hp) for hp in range(KC)] + [(1, hp) for hp in range(KC)]
        during_block = [[] for _ in blocks]
        # V for slot 1 drains during slot0 hp0/hp1
        for st in range(NT[1]):
            during_block[st % 2].extend(v_units(1, st))
        for j in range(1, len(blocks)):
            b, hp = blocks[j]
            for dst, wname in ((QT[b], "wq"), (KT[b], "wk")):
                for qs, w in QCH[b]:
                    during_block[j - 1].extend(
                        qtkt_units(b, hp, dst, wname, qs, w)
                    )
        # slot-0 output projection rides along slot-1's attention blocks
        s1_blocks = list(range(KC, 2 * KC))
        d0_units = [u for st in range(NT[0]) for u in outproj_units(0, st)]
        per_block = -(-len(d0_units) // len(s1_blocks))
        for i, j in enumerate(s1_blocks):
            during_block[j].extend(d0_units[i * per_block : (i + 1) * per_block])

        filler: list = []
        for i, (b, hp) in enumerate(blocks):
            filler.extend(during_block[i])
            iters_left = [len(QCH[b]) * NT[b]]
            for qs, w in QCH[b]:
                attn_chunk(b, hp, qs, w, filler, iters_left)
            while filler:
                filler.pop(0)()

        # slot-1 output projection (tail)
        for st in range(NT[1]):
            for u in outproj_units(1, st):
                u()

    nc.compile()
    return nc


def _get_program(bounds: tuple[int, int]):
    key = bounds
    if key not in _BUILD_CACHE:
        _BUILD_CACHE[key] = _build_bass(bounds)
    return _BUILD_CACHE[key]


def _xt_fp16(x16_seq, bound):
    """[S, D] fp16 rows -> feature-major [128, 4, bound] fp16."""
    xt = np.zeros((P, KC, bound), dtype=np.float16)
    n = x16_seq.shape[0]
    use = min(n, bound)
    # [use, 512] -> [512, use] -> [4, 128, use] -> [128, 4, use]
    t = x16_seq[:use].T.reshape(KC, P, use).transpose(1, 0, 2)
    xt[:, :, :use] = t
    return xt


def kernel(x, seq_lens, Wq, Wk, Wv, Wo, bo) -> np.ndarray:
    from concourse.bass_utils import run_bass_kernel_spmd

    x = np.ascontiguousarray(np.asarray(x, dtype=np.float32))
    seq_lens_np = np.asarray(seq_lens, dtype=np.int32)

    def prep_w(W):
        return np.ascontiguousarray(
            np.asarray(W, dtype=np.float16).reshape(KC, P, D).transpose(1, 0, 2)
        )

    w_in = {
        "wq": prep_w(Wq), "wk": prep_w(Wk), "wv": prep_w(Wv), "wo": prep_w(Wo)
    }
    bo32 = np.ascontiguousarray(np.asarray(bo, dtype=np.float32))
    x16 = np.asarray(x, dtype=np.float16)

    # Sort sequences by length: longest 8 -> slot 0, rest -> slot 1.
    order = np.argsort(-seq_lens_np, kind="stable")
    slot_seqs = [order[:N_CORES], order[N_CORES:]]
    bounds = tuple(int(_ceil128(seq_lens_np[s].max())) for s in slot_seqs)

    nc = _get_program(bounds)

    # Per-partition masks laid out as [slot, p, tile]: position t*128+p.
    pos = (np.arange(NT_MAX)[None, :] * P + np.arange(P)[:, None]).astype(np.int32)
    in_maps = []
    for c in range(N_CORES):
        seq_pair = [int(slot_seqs[0][c]), int(slot_seqs[1][c])]
        kbias = np.zeros((2, P, NT_MAX), dtype=np.float32)
        im = {"kbias": kbias, "bo": bo32, **w_in}
        for slot, seq in enumerate(seq_pair):
            valid = pos < int(seq_lens_np[seq])
            kbias[slot] = np.where(valid, 0.0, -60.0)
            im[f"xt{slot}"] = _xt_fp16(x16[seq], bounds[slot])
        in_maps.append(im)

    trace = bool(int(os.environ.get("KERNEL_TRACE", "0")))
    res = run_bass_kernel_spmd(
        nc, in_maps, core_ids=list(range(N_CORES)), trace=trace
    )
    kernel.last_results = res

    out = np.zeros((B, S, D), dtype=np.float32)
    for c in range(N_CORES):
        for slot in (0, 1):
            seq = int(slot_seqs[slot][c])
            L = int(seq_lens_np[seq])
            out[seq, :L] = res.results[c]["out"][slot][:L].astype(np.float32)
    return out



# revision 2
# speedup vs baseline: 1.1877x; 1.1877x over previous
"""Ragged-sequence multi-head attention (B=16, S=1024, D=512, H=8, DH=64)
for 8 Trainium2 NeuronCores.

Strategy v2: shard by (head-pair x sequence-group). The 16 sequences are
sorted by length and paired (1st+2nd, 3rd+4th, ...); each pair defines one
"slot" whose static k-tile count is the max of the two -> a common slot
profile shared by both groups. Group A takes the first of each pair, group
B the second. Cores = 4 head-pairs x 2 groups; every core runs the SAME
SPMD program over its group's packed sequences, computing Q/K/V, ragged
attention (per-slot loop bounds = that slot's k-tile count), and a
TRANSPOSED partial output projection with its head-pair's Wo slice. The
host sums the 4 per-pair partials of each group and adds bo.

vs the 2-slot baseline this removes ~45% of the attention area (each slot
pays only its own length, not the global max) and ~35% of projection work.

Per-core pipeline:
  1. QT/KT [128=2x64 dims, C] via stationary Wq/Wk pair-slices (4 kc chunks)
  2. V [128 keys, kt, 128 dims] via stationary xT k-tiles, moving Wv slice
  3. per slot, per q-chunk, per k-tile:
       scoresT[k, q] = K^T q (row-packed head pair)
       expT = exp(0.125*scoresT + key_mask_bias)   (ACT)
       outT[d, q] += V^T expT, denom += 1^T expT   (col-packed pairs)
  4. attnT = outT * reciprocal(denom)              (DVE)
  5. partial^T[o, q] = Wo_chunk^T @ attnT (4 persistent 128-col stationaries)
"""

import math
import os

import numpy as np

B, S, D = 16, 1024, 512
H, DH = 8, 64
N_CORES = 8
P = 128
KC = D // P  # 4 contraction chunks of 128
N_PAIRS = 4  # head pairs
N_GROUPS = 2  # sequence groups
N_SLOTS = B // N_GROUPS  # 8 slots per group

_BUILD_CACHE: dict = {}


def _qchunks(nt: int) -> list[int]:
    """Split nt 128-col tiles into balanced chunks of <=4 tiles each."""
    n = -(-nt // 4)
    base, rem = divmod(nt, n)
    return [(base + (1 if i < rem else 0)) * P for i in range(n)]


def _build_bass(profile: tuple[int, ...]):
    """Build the SPMD Bass program for a slot profile (k-tile counts)."""
    from contextlib import ExitStack

    import concourse.bass as bass
    import concourse.mybir as mybir
    import concourse.tile as tile
    from concourse import bacc

    fp32 = mybir.dt.float32
    fp16 = mybir.dt.float16
    Exp = mybir.ActivationFunctionType.Exp
    mult = mybir.AluOpType.mult

    NT = sum(profile)  # total k-tiles
    C = NT * P  # packed columns
    toff = [0] * len(profile)  # slot -> first global k-tile
    for j in range(1, len(profile)):
        toff[j] = toff[j - 1] + profile[j - 1]

    nc = bacc.Bacc("TRN2", target_bir_lowering=False, debug=False)

    xt_d = nc.dram_tensor("xt", [P, KC, C], fp16, kind="ExternalInput").ap()
    w_d = {
        name: nc.dram_tensor(name, [P, KC, P], fp16, kind="ExternalInput").ap()
        for name in ("wq", "wk", "wv", "wo")
    }
    kbias_d = nc.dram_tensor("kbias", [P, NT], fp32, kind="ExternalInput").ap()
    outp_d = nc.dram_tensor("outp", [P, KC, C], fp16, kind="ExternalOutput").ap()

    with ExitStack() as ctx:
        tc = ctx.enter_context(tile.TileContext(nc))
        singles = ctx.enter_context(tc.tile_pool(name="singles", bufs=1))
        big = ctx.enter_context(tc.tile_pool(name="big", bufs=1))
        epool = ctx.enter_context(tc.tile_pool(name="epool", bufs=3))
        opool = ctx.enter_context(tc.tile_pool(name="opool", bufs=4))
        mmps = ctx.enter_context(tc.tile_pool(name="mmps", bufs=2, space="PSUM"))
        scps = ctx.enter_context(tc.tile_pool(name="scps", bufs=2, space="PSUM"))
        accps = ctx.enter_context(tc.tile_pool(name="accps", bufs=1, space="PSUM"))

        ones64 = singles.tile([P, DH], fp16)
        nc.vector.memset(ones64, 1.0)
        warm = singles.tile([P, 512], fp16)
        nc.vector.memset(warm, 0.0)

        w_sb = {
            name: singles.tile([P, KC, P], fp16, name=f"w_{name}")
            for name in ("wq", "wk", "wv", "wo")
        }
        kbias_sb = singles.tile([P, NT], fp32)

        # ---- input DMA: weights on sync queue, x^T chunks on gpsimd ----
        nc.sync.dma_start(out=w_sb["wq"], in_=w_d["wq"])
        nc.sync.dma_start(out=w_sb["wk"], in_=w_d["wk"])
        xT = big.tile([P, KC, C], fp16, name="xT", tag="xT")
        for cs in range(0, C, 512):
            w = min(512, C - cs)
            nc.gpsimd.dma_start(out=xT[:, :, cs : cs + w], in_=xt_d[:, :, cs : cs + w])
        nc.sync.dma_start(out=w_sb["wv"], in_=w_d["wv"])
        nc.sync.dma_start(out=kbias_sb, in_=kbias_d)
        nc.sync.dma_start(out=w_sb["wo"], in_=w_d["wo"])

        # ---- PE/ACT warm-up while input DMA streams (no data deps) ----
        act_dummy = singles.tile([P, 2], fp32)
        nc.scalar.activation(act_dummy[:, 0:1], warm[:, 0:1], Exp, scale=1.0)
        for i in range(30):
            wps = mmps.tile([P, 512], fp32, name="warm_ps", tag="mm")
            nc.tensor.matmul(wps[0:DH, :], ones64, warm, start=True, stop=True)

        QT = big.tile([P, C], fp16, name="QT", tag="QT")
        KT = big.tile([P, C], fp16, name="KT", tag="KT")
        V = big.tile([P, NT, P], fp16, name="V", tag="V")
        attnT = big.tile([P, C], fp16, name="attnT", tag="attnT")

        def qk_units(dst, wname, qs, w):
            """Project x cols [qs, qs+w) with the pair's Wq/Wk slice."""
            ps_box = []

            def mk_mm(kc):
                def emit():
                    if not ps_box:
                        ps_box.append(mmps.tile([P, 512], fp32, name="qk_ps", tag="mm"))
                    nc.tensor.matmul(
                        ps_box[0][:, :w],
                        w_sb[wname][:, kc, :],
                        xT[:, kc, qs : qs + w],
                        start=(kc == 0),
                        stop=(kc == KC - 1),
                    )
                return emit

            def fin():
                # half-partition copies so RAW deps match score matmul reads
                nc.vector.tensor_copy(out=dst[0:DH, qs : qs + w], in_=ps_box[0][0:DH, :w])
                nc.vector.tensor_copy(out=dst[DH:P, qs : qs + w], in_=ps_box[0][DH:P, :w])

            return [mk_mm(kc) for kc in range(KC)] + [fin]

        def v_units(gkt):
            """V tile for global k-tile gkt: [128 keys, 128 pair dims]."""
            ps_box = []

            def mk_mm(kc):
                def emit():
                    if not ps_box:
                        ps_box.append(mmps.tile([P, 512], fp32, name="v_ps", tag="mm"))
                    nc.tensor.matmul(
                        ps_box[0][:, :P],
                        xT[:, kc, gkt * P : (gkt + 1) * P],
                        w_sb["wv"][:, kc, :],
                        start=(kc == 0),
                        stop=(kc == KC - 1),
                    )
                return emit

            def fin():
                nc.vector.tensor_copy(out=V[:, gkt, :], in_=ps_box[0][:, :P])

            return [mk_mm(kc) for kc in range(KC)] + [fin]

        def o_units(qs, w):
            """Transposed partial out-proj for q cols [qs, qs+w)."""
            units = []
            for oc in range(KC):
                def mk(oc):
                    ps_box = []

                    def emit_mm():
                        ps_box.append(mmps.tile([P, 512], fp32, name="o_ps", tag="mm"))
                        nc.tensor.matmul(
                            ps_box[0][:, :w],
                            w_sb["wo"][:, oc, :],
                            attnT[:, qs : qs + w],
                            start=True,
                            stop=True,
                        )

                    def emit_fin():
                        fout = opool.tile([P, 512], fp16, tag="fout")
                        nc.vector.tensor_copy(out=fout[:, :w], in_=ps_box[0][:, :w])
                        nc.sync.dma_start(
                            out=outp_d[:, oc, qs : qs + w], in_=fout[:, :w]
                        )

                    return [emit_mm, emit_fin]

                units.extend(mk(oc))
            return units

        def attn_chunk(j, qs, w, filler, iters_left):
            """Attention for slot j, q cols [qs, qs+w), both heads of pair."""
            nt = profile[j]
            o_ps = accps.tile([P, 512], fp32, name="opv_ps", tag="opv_ps")
            d_ps = accps.tile([P, 512], fp32, name="d_ps", tag="d_ps")

            def emit_scores_exp(kt):
                gkt = toff[j] + kt
                ks = gkt * P
                s_pair = scps.tile([P, 1024], fp32, name="s_pair", tag="s_pair")
                nc.tensor.matmul(
                    s_pair[:, 0:w],
                    KT[0:DH, ks : ks + P],
                    QT[0:DH, qs : qs + w],
                    start=True, stop=True, tile_position=(0, 0),
                )
                nc.tensor.matmul(
                    s_pair[:, 512 : 512 + w],
                    KT[DH:P, ks : ks + P],
                    QT[DH:P, qs : qs + w],
                    start=True, stop=True, tile_position=(DH, 0),
                )
                e_pair = epool.tile([P, 2, 512], fp16, name="e_pair", tag="e_pair")
                nc.scalar.activation(
                    e_pair[:, :, :w],
                    s_pair.rearrange("p (h q) -> p h q", h=2)[:, :, :w],
                    Exp, bias=kbias_sb[:, gkt : gkt + 1], scale=DH**-0.5,
                )
                return e_pair

            def emit_pv(kt, e_pair):
                gkt = toff[j] + kt
                first, last = kt == 0, kt == nt - 1
                nc.tensor.matmul(
                    o_ps[0:DH, :w], V[:, gkt, 0:DH],
                    e_pair[:, 0, :w], start=first, stop=last,
                    tile_position=(0, 0), skip_group_check=True,
                )
                nc.tensor.matmul(
                    o_ps[DH:P, :w], V[:, gkt, DH:P],
                    e_pair[:, 1, :w], start=first, stop=last,
                    tile_position=(0, DH), skip_group_check=True,
                )
                nc.tensor.matmul(
                    d_ps[0:DH, :w], ones64, e_pair[:, 0, :w],
                    start=first, stop=last,
                    tile_position=(0, 0), skip_group_check=True,
                )
                nc.tensor.matmul(
                    d_ps[DH:P, :w], ones64, e_pair[:, 1, :w],
                    start=first, stop=last,
                    tile_position=(0, DH), skip_group_check=True,
                )

            pending = None
            for kt in range(nt):
                e_pair = emit_scores_exp(kt)
                if pending is not None:
                    emit_pv(*pending)
                pending = (kt, e_pair)
                if filler and iters_left[0] > 0:
                    k = -(-len(filler) // iters_left[0])
                    for _ in range(min(k, len(filler))):
                        filler.pop(0)()
                iters_left[0] -= 1
            emit_pv(*pending)
            rrep = epool.tile([P, 512], fp32, tag="rrep", bufs=2)
            nc.vector.reciprocal_approx_fast(out=rrep[:, :w], in_=d_ps[:, :w])
            nc.vector.tensor_tensor(
                attnT[:, qs : qs + w], o_ps[:, :w], rrep[:, :w], mult
            )

        # ---- choreographed emission ----
        # slot j's attention blocks absorb filler: QK/V of slot j+1 and
        # out-proj of already-finished chunks.
        chunks = []  # (slot, qs, w)
        for j, nt in enumerate(profile):
            qs = toff[j] * P
            for w in _qchunks(nt):
                chunks.append((j, qs, w))
                qs += w

        def slot_prep_units(j):
            us = []
            qs = toff[j] * P
            for w in _qchunks(profile[j]):
                us.extend(qk_units(QT, "wq", qs, w))
                us.extend(qk_units(KT, "wk", qs, w))
                qs += w
            for kt in range(profile[j]):
                us.extend(v_units(toff[j] + kt))
            return us

        # slot 0 prep emitted directly (nothing to overlap it with yet)
        for u in slot_prep_units(0):
            u()

        filler: list = []
        next_prep = 1
        for j, qs, w in chunks:
            if next_prep == j + 1 and next_prep < len(profile):
                filler.extend(slot_prep_units(next_prep))
                next_prep += 1
            iters_left = [profile[j]]
            attn_chunk(j, qs, w, filler, iters_left)
            filler.extend(o_units(qs, w))
        while filler:
            filler.pop(0)()

    nc.compile()
    return nc


def _get_program(profile: tuple[int, ...]):
    if profile not in _BUILD_CACHE:
        _BUILD_CACHE[profile] = _build_bass(profile)
    return _BUILD_CACHE[profile]


def kernel(x, seq_lens, Wq, Wk, Wv, Wo, bo) -> np.ndarray:
    from concourse.bass_utils import run_bass_kernel_spmd

    x = np.asarray(x, dtype=np.float32)
    seq_lens_np = np.asarray(seq_lens, dtype=np.int32)
    x16 = np.asarray(x, dtype=np.float16)

    nt = np.maximum(1, -(-seq_lens_np // P)).astype(np.int64)
    order = np.argsort(-seq_lens_np, kind="stable")
    pairs = [(int(order[2 * i]), int(order[2 * i + 1])) for i in range(N_SLOTS)]
    profile = tuple(int(max(nt[a], nt[b])) for a, b in pairs)
    groups = [[a for a, b in pairs], [b for a, b in pairs]]
    NT = sum(profile)
    C = NT * P
    toff = np.concatenate([[0], np.cumsum(profile)])[:-1]

    nc = _get_program(profile)

    # per-pair weight slices: [128, KC, 128] fp16, kc-major partition layout
    def slice_w_in(W, p):  # W[:, p*128:(p+1)*128] -> [128, 4, 128]
        ws = np.asarray(W, dtype=np.float16)[:, p * P : (p + 1) * P]
        return np.ascontiguousarray(ws.reshape(KC, P, P).transpose(1, 0, 2))

    def slice_wo(W, p):  # Wo[p*128:(p+1)*128, :] -> [128 d, 4 oc, 128 o]
        ws = np.asarray(W, dtype=np.float16)[p * P : (p + 1) * P, :]
        return np.ascontiguousarray(ws.reshape(P, KC, P).transpose(0, 1, 2))

    w_pair = [
        {
            "wq": slice_w_in(Wq, p),
            "wk": slice_w_in(Wk, p),
            "wv": slice_w_in(Wv, p),
            "wo": slice_wo(Wo, p),
        }
        for p in range(N_PAIRS)
    ]

    # per-group packed x^T and key-mask bias
    pos = np.arange(P, dtype=np.int32)
    g_xt, g_kb = [], []
    for g in range(N_GROUPS):
        xt = np.zeros((P, KC, C), dtype=np.float16)
        kb = np.full((P, NT), -60.0, dtype=np.float32)
        for j, s in enumerate(groups[g]):
            L = int(seq_lens_np[s])
            cs = int(toff[j]) * P
            t = x16[s, :L].T.reshape(KC, P, L).transpose(1, 0, 2)
            xt[:, :, cs : cs + L] = t
            for kt in range(profile[j]):
                valid = (kt * P + pos) < L
                kb[:, int(toff[j]) + kt] = np.where(valid, 0.0, -60.0)
        g_xt.append(xt)
        g_kb.append(kb)

    in_maps = []
    for c in range(N_CORES):
        g, p = c // N_PAIRS, c % N_PAIRS
        in_maps.append({"xt": g_xt[g], "kbias": g_kb[g], **w_pair[p]})

    trace = bool(int(os.environ.get("KERNEL_TRACE", "0")))
    res = run_bass_kernel_spmd(
        nc, in_maps, core_ids=list(range(N_CORES)), trace=trace
    )
    kernel.last_results = res

    bo32 = np.asarray(bo, dtype=np.float32)
    out = np.zeros((B, S, D), dtype=np.float32)
    for g in range(N_GROUPS):
        acc = np.zeros((P, KC, C), dtype=np.float32)
        for p in range(N_PAIRS):
            acc += res.results[g * N_PAIRS + p]["outp"].astype(np.float32)
        # acc[op, oc, q] -> out[q, oc*128+op]
        acc = acc.transpose(2, 1, 0).reshape(C, D)
        for j, s in enumerate(groups[g]):
            L = int(seq_lens_np[s])
            cs = int(toff[j]) * P
            out[s, :L] = acc[cs : cs + L] + bo32
    return out
